# revision 1
# baseline (speedup 1.0000x reference)
"""Trainium2 Bass kernel for nn_ClusterLoss_Regr (topk_masking).

Computes  mean_b(128 - max_p((128 - d[b,p]) * [|proto[p] - label[b]| <= 0.5]))
for d: [8192, 4096] f32, labels: [8192] f32, proto: [4096] f32 -> scalar f32.

Sharding: data-parallel over the batch axis across 8 NeuronCores (1024 rows
per core); proto_classes replicated; final mean on host.

Device work per core (memory-bound, 16 MiB HBM read):
  - proto row DMA'd once and broadcast to all 128 partitions (GPSIMD).
  - 8 row-tiles of [128, 4096]; each processed by ONE fused custom-DVE
    instruction:
        out[p,k]  = select((proto[k] - label[p])^2 <= 0.25, 128 - d[p,k], 0)
        accum[p]  = max_k out[p,k]
    which is bit-exact with the reference's mask/multiply/top_k(1) chain
    (f32 monotone rounding: |x| <= 0.5  <=>  fl(x^2) <= 0.25).
  - raw Bass with manual semaphores (no Tile exit drain/barrier): head DMAs
    (proto, labels) ride the scalar HWDGE ring so d-tile DMAs stream
    back-to-back on the sync ring; DVE op t is gated only on d-tile t.
Host: gather [8192] row maxima, loss = mean(128 - rowmax)  (f32 rounding
matches the reference exactly; mean accumulated in f64, cast to f32).
"""

import numpy as np

B, P = 8192, 4096
NCORES = 8
BSH = B // NCORES  # 1024 rows per core
RT = BSH // 128    # 8 row-tiles of 128 rows
MAX_DIST = np.float32(128.0)

_cache: dict = {}


def _ensure_path():
    try:
        import concourse.bass  # noqa: F401
    except ImportError:
        import sys

        for p in ("/opt/trn_rl_repo",):
            if p not in sys.path:
                sys.path.insert(0, p)


def _register_dve_op():
    """Register the fused mask+invert+rowmax op in the custom-DVE registry.

    Idempotent; computes its own uops_sha so no golden file is needed.
    """
    from concourse import dve_ops
    from concourse.dve_spec import (
        C0,
        C1,
        C2,
        Spec,
        Src0,
        Src1,
        Zero,
        lower,
        maxx,
        select,
        sq,
    )
    from concourse.dve_uop import DveOpSpec

    name = "CLUSTER_MASK_MAX_ANT"
    for op in dve_ops.OPS:
        if op.name == name:
            return op

    def _ref(in0, in1, s0, s1, imm2):
        o = np.where(
            (in1.astype(np.float32) - s0) ** 2 <= imm2,
            (np.float32(s1) - in0).astype(np.float32),
            np.float32(0.0),
        ).astype(np.float32)
        return o, o.max(axis=-1, keepdims=True)

    spec = Spec(
        body=select(sq(Src1 - C0) <= C2, C1 - Src0, Zero),
        accum=maxx,
        accum_init=Zero,
        reference=_ref,
    )
    shas: dict = {}
    op = dve_ops.DveOp(name, spec, subdim=False, uops_sha=shas)
    dve_ops.OPS.append(op)
    row = dve_ops._CUSTOM_DVE_ROW_BASE + len(dve_ops.OPS) - 1
    dve_ops._SUB_OPCODE_FOR_NAME[name] = row
    dve_ops.CUSTOM_DVE_SPECS[name] = spec
    for ver in ("v3", "v4"):
        shas[ver] = DveOpSpec(
            name=name, opcode=row, uops=lower(spec, ver=ver), rd1_en=True
        ).sha(ver)
    return op


def _get_bass():
    if "nc" in _cache:
        return _cache["nc"]
    _ensure_path()
    import concourse.bacc as bacc
    import concourse.mybir as mybir

    op = _register_dve_op()
    f32 = mybir.dt.float32
    nc = bacc.Bacc(
        "TRN2", target_bir_lowering=False, debug=False, num_devices=NCORES
    )
    d_ap = nc.dram_tensor("d", [BSH, P], f32, kind="ExternalInput").ap()
    lab_ap = nc.dram_tensor("labels_col", [128, RT], f32, kind="ExternalInput").ap()
    proto_ap = nc.dram_tensor("proto", [P], f32, kind="ExternalInput").ap()
    # The LAST row-tile is processed as NSPLIT quarter-width ops so the
    # final DVE op rides only a quarter tile behind the last DMA byte.
    # stats columns: [0..RT-2] = tiles 0..RT-2, [RT-1..RT+2] = quarters of
    # tile RT-1.
    NSPLIT = 4
    NCOLS = RT - 1 + NSPLIT
    out_ap = nc.dram_tensor("rowmax", [128, NCOLS], f32, kind="ExternalOutput").ap()

    prow = nc.alloc_sbuf_tensor("prow", [1, P], f32).ap()
    proto_tile = nc.alloc_sbuf_tensor("proto_tile", [128, P], f32).ap()
    labels_tile = nc.alloc_sbuf_tensor("labels_tile", [128, RT], f32).ap()
    stats = nc.alloc_sbuf_tensor("stats", [128, NCOLS], f32).ap()
    scratch = nc.alloc_sbuf_tensor("scratch", [128, P], f32).ap()
    d_tiles = [nc.alloc_sbuf_tensor(f"dt{t}", [128, P], f32).ap() for t in range(RT)]

    H = P // NSPLIT           # split width of the last tile
    ND = RT - 1 + NSPLIT      # number of d DMAs
    NV = ND                   # number of DVE ops

    # (tile, col_lo, width, stats_col) in stream order
    work = [(t, 0, P, t) for t in range(RT - 1)]
    for s in range(NSPLIT):
        work.append((RT - 1, s * H, H, RT - 1 + s))

    # One semaphore per DMA. A DMA's semaphore is bumped +1 by EACH of the
    # 16 SDMA engines as it finishes its share, so a shared semaphore with
    # cumulative thresholds can fire before an individual DMA has fully
    # landed when engines skew. A dedicated sem == 16 is exact.
    d_sems = [nc.alloc_semaphore(f"d_sem{i}") for i in range(ND)]
    prow_sem = nc.alloc_semaphore("prow_sem")
    lab_sem = nc.alloc_semaphore("lab_sem")
    out_sem = nc.alloc_semaphore("out_sem")
    pb_sem = nc.alloc_semaphore("pb_sem")
    dve_sem = nc.alloc_semaphore("dve_sem")

    with nc.Block() as block:

        @block.scalar
        def _(scalar):
            scalar.dma_start(prow[:], proto_ap[None, :]).then_inc(prow_sem, 16)
            scalar.dma_start(labels_tile[:], lab_ap[:]).then_inc(lab_sem, 16)

        @block.sync
        def _(sync):
            for i, (t, lo, w, _col) in enumerate(work):
                sync.dma_start(
                    d_tiles[t][:, lo : lo + w],
                    d_ap[128 * t : 128 * (t + 1), lo : lo + w],
                ).then_inc(d_sems[i], 16)
            sync.wait_ge(dve_sem, NV)
            sync.dma_start(out_ap[:], stats[:]).then_inc(out_sem, 16)
            sync.wait_ge(out_sem, 16)
            # Reset all kernel semaphores so re-executing the loaded NEFF
            # behaves identically to the first run. Every consumer has
            # finished its waits by this point (dve_sem >= NV implies vector
            # and gpsimd are done; out_sem >= 16 implies all DMAs landed).
            all_sems = sorted(
                s.num
                for s in [*d_sems, prow_sem, lab_sem, out_sem, pb_sem, dve_sem]
            )
            lo = prev = all_sems[0]
            for n in all_sems[1:] + [None]:
                if n is not None and n == prev + 1:
                    prev = n
                    continue
                sync.sem_clear(range(lo, prev + 1))
                if n is not None:
                    lo = prev = n

        @block.gpsimd
        def _(gpsimd):
            gpsimd.wait_ge(prow_sem, 16)
            gpsimd.partition_broadcast(proto_tile[:], prow[:]).then_inc(pb_sem, 1)

        @block.vector
        def _(vector):
            vector.wait_ge(pb_sem, 1)
            vector.wait_ge(lab_sem, 16)
            for i, (t, lo, w, col) in enumerate(work):
                vector.wait_ge(d_sems[i], 16)
                nc.vector._custom_dve(
                    op,
                    out=scratch[:, :w],
                    in0=d_tiles[t][:, lo : lo + w],
                    in1=proto_tile[:, lo : lo + w],
                    s0=labels_tile[:, t : t + 1],
                    s1=float(MAX_DIST),
                    imm2=0.25,
                    accum_out=stats[:, col : col + 1],
                ).then_inc(dve_sem, 1)

    nc.compile()
    _cache["nc"] = nc
    return nc


def _run_device(min_distances, labels, proto_classes, trace=False):
    nc = _get_bass()
    from concourse.bass_utils import run_bass_kernel_spmd

    proto = np.ascontiguousarray(np.asarray(proto_classes, dtype=np.float32))
    in_maps = []
    for c in range(NCORES):
        dsh = np.ascontiguousarray(
            np.asarray(min_distances[c * BSH : (c + 1) * BSH], dtype=np.float32)
        )
        lsh = np.ascontiguousarray(
            np.asarray(labels[c * BSH : (c + 1) * BSH], dtype=np.float32)
            .reshape(RT, 128)
            .T
        )
        in_maps.append({"d": dsh, "labels_col": lsh, "proto": proto})
    return run_bass_kernel_spmd(
        nc, in_maps, core_ids=list(range(NCORES)), trace=trace
    )


def kernel(min_distances, labels, proto_classes):
    res = _run_device(min_distances, labels, proto_classes).results
    # stats columns: [0..RT-2] = tiles 0..RT-2, [RT-1..] = quarter maxima of
    # tile RT-1 (combine by max). Row = 1024*c + 128*t + p.
    stats = np.stack([np.asarray(res[c]["rowmax"]) for c in range(NCORES)])
    t_last = stats[:, :, RT - 1 :].max(axis=2)
    rowmax = np.concatenate(
        [stats[:, :, : RT - 1], t_last[:, :, None]], axis=2
    )
    rowmax = rowmax.transpose(0, 2, 1).reshape(B).astype(np.float32)
    loss_rows = (MAX_DIST - rowmax).astype(np.float32)
    return np.array(loss_rows.mean(dtype=np.float64), dtype=np.float32)

